# revision 2
# baseline (speedup 1.0000x reference)
"""Discrete transfer function (2nd-order IIR, Butterworth lowpass) over T=2^23
samples, on 8 TRN2 NeuronCores.

Approach: the filter is stable (poles at radius ~0.577), so the IIR's impulse
response decays below float32 precision within ~100 taps.  The whole recurrence
y = filter(b, a, r), shifted by one sample and scaled by dt, is therefore
exactly (to f32 precision) a 128-tap FIR convolution y[t] = sum_n g[n] r[t-n],
with g derived from (b, a) on the host in float64.

The convolution maps onto the TensorEngine as two banded-Toeplitz matmuls:
with R[k, j] = r[128 j + k] (a [128, J] reshape of the signal),
    Y[c, j] = sum_k W0[k, c] R[k, j] + sum_k W1[k, c] R[k, j-1]
where W0[k, c] = g[c - k] (lower-triangular band) and W1[k, c] = g[c - k + 128]
(strictly upper triangle).  Each core gets a contiguous 2^20-sample chunk plus
a 128-sample halo column from its left neighbor.  The host does the (free)
reshape/transpose so every DMA is 128 partitions x contiguous rows.

Raw bass (not Tile): the pipeline is static and simple — input DMAs on the SP
HWDGE ring, matmuls on PE, PSUM->SBUF copies on DVE, output DMAs on the ACT
HWDGE ring, double-buffered with explicit semaphores.  (Tile was tried first
but attaches >1 embedded sync-wait to Matmult instructions, which the walrus
codegen rejects; standalone WAIT instructions have no such limit.)
"""

import numpy as np

import concourse.bass as bass
import concourse.mybir as mybir
from concourse.bass_utils import run_bass_kernel_spmd

N_CORES = 8
T = 8388608                 # 2**23, matches the fixed problem size
C = T // N_CORES            # samples per core
J = C // 128                # columns per core in the [128, J] layout
BLK = 2048                  # columns per pipeline block (1 MiB per DMA)
NBLK = J // BLK
L = 128                     # FIR taps kept (g[n] ~ 0.577^n -> ~1e-30 at n=127)
DT = 1.0                    # module's dt (hardcoded in the source nn.Module)

# Matmul operand dtype: float32 is exact; float32r runs the PE 4x faster at
# ~TF32-ish precision.
MM_DT = mybir.dt.float32

_PROGRAM_CACHE: dict = {}

# test.py reads this after calling kernel(..., _trace=True)
LAST_RESULTS = None


def _fir_coeffs(b: np.ndarray, a: np.ndarray) -> np.ndarray:
    """g[n] (length L, float64) with y_out[t] = sum_n g[n] r[t-n].

    g folds together: the IIR impulse response of b/a, the module's
    one-sample output delay, and the dt scale."""
    b = np.asarray(b, np.float64)
    a = np.asarray(a, np.float64)
    a = a / a[0]
    h = np.zeros(L, np.float64)
    for n in range(L):
        acc = b[n] if n < len(b) else 0.0
        for j in range(1, len(a)):
            if n - j >= 0:
                acc -= a[j] * h[n - j]
        h[n] = acc
    g = np.zeros(L, np.float64)
    g[1:] = DT * h[: L - 1]  # y_out[t] = dt * y_iir[t-1], y_out[0] = 0
    return g


def _weight_mats(g: np.ndarray) -> tuple[np.ndarray, np.ndarray]:
    g32 = g.astype(np.float32)
    idx_c = np.arange(128)[None, :]
    idx_k = np.arange(128)[:, None]
    d = idx_c - idx_k  # c - k
    W0 = np.where(d >= 0, g32[np.clip(d, 0, L - 1)], 0.0).astype(np.float32)
    W1 = np.where(d < 0, g32[np.clip(d + 128, 0, L - 1)], 0.0).astype(np.float32)
    return W0, W1


def _build_program(mm_dt):
    f32 = mybir.dt.float32
    nc = bass.Bass()
    R = nc.dram_tensor("R", [128, J + 1], mm_dt, kind="ExternalInput")
    W0 = nc.dram_tensor("W0", [128, 128], mm_dt, kind="ExternalInput")
    W1 = nc.dram_tensor("W1", [128, 128], mm_dt, kind="ExternalInput")
    Y = nc.dram_tensor("Y", [128, J], f32, kind="ExternalOutput")

    with (
        nc.sbuf_tensor([128, 128], mm_dt) as w0t,
        nc.sbuf_tensor([128, 128], mm_dt) as w1t,
        nc.sbuf_tensor([128, BLK + 1], mm_dt) as rin0,
        nc.sbuf_tensor([128, BLK + 1], mm_dt) as rin1,
        nc.sbuf_tensor([128, BLK], f32) as yout0,
        nc.sbuf_tensor([128, BLK], f32) as yout1,
        nc.psum_tensor([128, BLK], f32) as pt0,
        nc.psum_tensor([128, BLK], f32) as pt1,
        nc.semaphore("s_in") as s_in,
        nc.semaphore("s_pe") as s_pe,
        nc.semaphore("s_cp") as s_cp,
        nc.semaphore("s_out") as s_out,
        nc.Block() as block,
    ):
        rins = [rin0, rin1]
        youts = [yout0, yout1]
        pts = [pt0, pt1]

        @block.sync
        def _(sync):
            sync.dma_start(out=w0t[:, :], in_=W0[:, :]).then_inc(s_in, 16)
            sync.dma_start(out=w1t[:, :], in_=W1[:, :]).then_inc(s_in, 16)
            for b_ in range(NBLK):
                if b_ >= 2:
                    # rin slot reuse: matmuls of block b_-2 must be done
                    sync.wait_ge(s_pe, b_ - 1)
                sync.dma_start(
                    out=rins[b_ % 2][:, :],
                    in_=R[:, b_ * BLK : b_ * BLK + BLK + 1],
                ).then_inc(s_in, 16)

        @block.tensor
        def _(tensor):
            for b_ in range(NBLK):
                tensor.wait_ge(s_in, 48 + 16 * b_)  # weights + rin b_ loaded
                if b_ >= 2:
                    tensor.wait_ge(s_cp, b_ - 1)  # psum slot drained by copy
                pt = pts[b_ % 2]
                rin = rins[b_ % 2]
                for s in range(BLK // 512):
                    nc.tensor.matmul(
                        pt[:, s * 512 : (s + 1) * 512],
                        w0t[:, :],
                        rin[:, s * 512 + 1 : s * 512 + 513],
                        start=True,
                        stop=False,
                    )
                    mm = nc.tensor.matmul(
                        pt[:, s * 512 : (s + 1) * 512],
                        w1t[:, :],
                        rin[:, s * 512 : s * 512 + 512],
                        start=False,
                        stop=True,
                    )
                mm.then_inc(s_pe, 1)

        @block.vector
        def _(vector):
            for b_ in range(NBLK):
                vector.wait_ge(s_pe, b_ + 1)
                if b_ >= 2:
                    vector.wait_ge(s_out, 16 * (b_ - 1))  # yout slot flushed
                nc.vector.tensor_copy(
                    out=youts[b_ % 2][:, :], in_=pts[b_ % 2][:, :]
                ).then_inc(s_cp, 1)

        @block.scalar
        def _(scalar):
            for b_ in range(NBLK):
                scalar.wait_ge(s_cp, b_ + 1)
                scalar.dma_start(
                    out=Y[:, b_ * BLK : (b_ + 1) * BLK], in_=youts[b_ % 2][:, :]
                ).then_inc(s_out, 16)
            scalar.wait_ge(s_out, 16 * NBLK)

    return nc


def _get_program(mm_dt):
    key = str(mm_dt)
    if key not in _PROGRAM_CACHE:
        _PROGRAM_CACHE[key] = _build_program(mm_dt)
    return _PROGRAM_CACHE[key]


def kernel(r, b, a, _trace: bool = False, **_trace_kwargs):
    global LAST_RESULTS
    r = np.ascontiguousarray(np.asarray(r, dtype=np.float32))
    assert r.shape == (T,), r.shape

    g = _fir_coeffs(b, a)
    W0, W1 = _weight_mats(g)

    in_maps = []
    for m in range(N_CORES):
        halo = (
            np.zeros(128, np.float32) if m == 0 else r[m * C - 128 : m * C]
        )
        rbuf = np.concatenate([halo, r[m * C : (m + 1) * C]])
        R = np.ascontiguousarray(rbuf.reshape(J + 1, 128).T)
        in_maps.append({"R": R, "W0": W0, "W1": W1})

    nc = _get_program(MM_DT)
    res = run_bass_kernel_spmd(
        nc, in_maps, core_ids=list(range(N_CORES)), trace=_trace, **_trace_kwargs
    )
    LAST_RESULTS = res

    y = np.concatenate(
        [
            np.ascontiguousarray(res.results[m]["Y"].T).reshape(-1)
            for m in range(N_CORES)
        ]
    )
    return y


# revision 7
# speedup vs baseline: 1.0916x; 1.0916x over previous
"""Discrete transfer function (2nd-order IIR, Butterworth lowpass) over T=2^23
samples, on 8 TRN2 NeuronCores.

Approach: the filter is stable (poles at radius ~0.577), so the IIR's impulse
response decays below float32 precision within ~100 taps.  The whole recurrence
y = filter(b, a, r), shifted by one sample and scaled by dt, is therefore
exactly (to f32 precision) a 128-tap FIR convolution y[t] = sum_n g[n] r[t-n],
with g derived from (b, a) on the host in float64.

The convolution maps onto the TensorEngine as two banded-Toeplitz matmuls:
with R[k, j] = r[128 j + k] (a [128, J] reshape of the signal),
    Y[c, j] = sum_k W0[k, c] R[k, j] + sum_k W1[k, c] R[k, j-1]
where W0[k, c] = g[c - k] (lower-triangular band) and W1[k, c] = g[c - k + 128]
(strictly upper triangle).  Each core gets a contiguous 2^20-sample chunk plus
a 128-sample halo column from its left neighbor.  The host does the (free)
reshape/transpose so every DMA is 128 partitions x contiguous rows.

Raw bass (not Tile): the pipeline is static and simple — input DMAs on the SP
HWDGE ring, matmuls on PE, PSUM->SBUF copies on DVE, output DMAs on the ACT
HWDGE ring, double-buffered with explicit semaphores.  (Tile was tried first
but attaches >1 embedded sync-wait to Matmult instructions, which the walrus
codegen rejects; standalone WAIT instructions have no such limit.)
"""

import numpy as np

import concourse.bass as bass
import concourse.mybir as mybir
from concourse.bass_utils import run_bass_kernel_spmd

N_CORES = 8
T = 8388608                 # 2**23, matches the fixed problem size
C = T // N_CORES            # samples per core
J = C // 128                # columns per core in the [128, J] layout
BLK = 1024                  # columns per pipeline block (512 KiB per DMA)
NBLK = J // BLK
L = 128                     # FIR taps kept (g[n] ~ 0.577^n -> ~1e-30 at n=127)
DT = 1.0                    # module's dt (hardcoded in the source nn.Module)
CP_SPLIT = 640              # PSUM->SBUF drain: DVE copies [0, CP_SPLIT), ACT the rest

# Matmul operand dtype: float32 is exact; float32r runs the PE 4x faster at
# ~TF32-ish precision.
MM_DT = mybir.dt.float32

_PROGRAM_CACHE: dict = {}

# test.py reads this after calling kernel(..., _trace=True)
LAST_RESULTS = None


def _fir_coeffs(b: np.ndarray, a: np.ndarray) -> np.ndarray:
    """g[n] (length L, float64) with y_out[t] = sum_n g[n] r[t-n].

    g folds together: the IIR impulse response of b/a, the module's
    one-sample output delay, and the dt scale."""
    b = np.asarray(b, np.float64)
    a = np.asarray(a, np.float64)
    a = a / a[0]
    h = np.zeros(L, np.float64)
    for n in range(L):
        acc = b[n] if n < len(b) else 0.0
        for j in range(1, len(a)):
            if n - j >= 0:
                acc -= a[j] * h[n - j]
        h[n] = acc
    g = np.zeros(L, np.float64)
    g[1:] = DT * h[: L - 1]  # y_out[t] = dt * y_iir[t-1], y_out[0] = 0
    return g


def _weight_mats(g: np.ndarray) -> tuple[np.ndarray, np.ndarray]:
    g32 = g.astype(np.float32)
    idx_c = np.arange(128)[None, :]
    idx_k = np.arange(128)[:, None]
    d = idx_c - idx_k  # c - k
    W0 = np.where(d >= 0, g32[np.clip(d, 0, L - 1)], 0.0).astype(np.float32)
    W1 = np.where(d < 0, g32[np.clip(d + 128, 0, L - 1)], 0.0).astype(np.float32)
    return W0, W1


def _build_program(mm_dt):
    f32 = mybir.dt.float32
    nc = bass.Bass()
    R = nc.dram_tensor("R", [128, J + 1], mm_dt, kind="ExternalInput")
    W0 = nc.dram_tensor("W0", [128, 128], mm_dt, kind="ExternalInput")
    W1 = nc.dram_tensor("W1", [128, 128], mm_dt, kind="ExternalInput")
    Y = nc.dram_tensor("Y", [128, J], f32, kind="ExternalOutput")

    NBUF = 4

    with (
        nc.sbuf_tensor([128, 128], mm_dt) as w0t,
        nc.sbuf_tensor([128, 128], mm_dt) as w1t,
        nc.sbuf_tensor([128, NBUF * (BLK + 1)], mm_dt) as rin_all,
        nc.sbuf_tensor([128, NBUF * BLK], f32) as yout_all,
        nc.psum_tensor([128, BLK], f32) as pt0,
        nc.psum_tensor([128, BLK], f32) as pt1,
        nc.psum_tensor([128, BLK], f32) as pt2,
        nc.psum_tensor([128, BLK], f32) as pt3,
        nc.semaphore("s_in") as s_in,
        nc.semaphore("s_pe") as s_pe,
        nc.semaphore("s_cpv") as s_cpv,
        nc.semaphore("s_cpa") as s_cpa,
        nc.semaphore("s_out") as s_out,
        nc.Block() as block,
    ):
        pts = [pt0, pt1, pt2, pt3]
        rin = [
            rin_all[:, i * (BLK + 1) : (i + 1) * (BLK + 1)] for i in range(NBUF)
        ]
        yout = [yout_all[:, i * BLK : (i + 1) * BLK] for i in range(NBUF)]

        @block.sync
        def _(sync):
            sync.dma_start(out=w0t[:, :], in_=W0[:, :]).then_inc(s_in, 16)
            sync.dma_start(out=w1t[:, :], in_=W1[:, :]).then_inc(s_in, 16)
            for b_ in range(NBLK):
                if b_ >= NBUF:
                    # rin slot reuse: matmuls of block b_-NBUF must be done
                    sync.wait_ge(s_pe, b_ - NBUF + 1)
                sync.dma_start(
                    out=rin[b_ % NBUF],
                    in_=R[:, b_ * BLK : b_ * BLK + BLK + 1],
                ).then_inc(s_in, 16)

        @block.tensor
        def _(tensor):
            for b_ in range(NBLK):
                tensor.wait_ge(s_in, 48 + 16 * b_)  # weights + rin b_ loaded
                if b_ >= NBUF:
                    # psum slot drained by both copy engines
                    tensor.wait_ge(s_cpv, b_ - NBUF + 1)
                    tensor.wait_ge(s_cpa, b_ - NBUF + 1)
                pt = pts[b_ % NBUF]
                ri = rin[b_ % NBUF]
                nsub = BLK // 512
                for s in range(nsub):
                    nc.tensor.matmul(
                        pt[:, s * 512 : (s + 1) * 512],
                        w0t[:, :],
                        ri[:, s * 512 + 1 : s * 512 + 513],
                        start=True,
                        stop=False,
                    )
                for s in range(nsub):
                    mm = nc.tensor.matmul(
                        pt[:, s * 512 : (s + 1) * 512],
                        w1t[:, :],
                        ri[:, s * 512 : s * 512 + 512],
                        start=False,
                        stop=True,
                    )
                mm.then_inc(s_pe, 1)

        @block.vector
        def _(vector):
            for b_ in range(NBLK):
                vector.wait_ge(s_pe, b_ + 1)
                if b_ >= NBUF:
                    vector.wait_ge(s_out, 16 * (b_ - NBUF + 1))  # yout flushed
                nc.vector.tensor_copy(
                    out=yout[b_ % NBUF][:, :CP_SPLIT],
                    in_=pts[b_ % NBUF][:, :CP_SPLIT],
                ).then_inc(s_cpv, 1)

        @block.scalar
        def _(scalar):
            for b_ in range(NBLK):
                scalar.wait_ge(s_pe, b_ + 1)
                if b_ >= NBUF:
                    scalar.wait_ge(s_out, 16 * (b_ - NBUF + 1))  # yout flushed
                nc.scalar.copy(
                    out=yout[b_ % NBUF][:, CP_SPLIT:],
                    in_=pts[b_ % NBUF][:, CP_SPLIT:],
                ).then_inc(s_cpa, 1)
                scalar.wait_ge(s_cpv, b_ + 1)
                scalar.dma_start(
                    out=Y[:, b_ * BLK : (b_ + 1) * BLK], in_=yout[b_ % NBUF]
                ).then_inc(s_out, 16)
            scalar.wait_ge(s_out, 16 * NBLK)

    return nc


def _get_program(mm_dt):
    key = str(mm_dt)
    if key not in _PROGRAM_CACHE:
        _PROGRAM_CACHE[key] = _build_program(mm_dt)
    return _PROGRAM_CACHE[key]


def kernel(r, b, a, _trace: bool = False, **_trace_kwargs):
    global LAST_RESULTS
    r = np.ascontiguousarray(np.asarray(r, dtype=np.float32))
    assert r.shape == (T,), r.shape

    g = _fir_coeffs(b, a)
    W0, W1 = _weight_mats(g)

    in_maps = []
    for m in range(N_CORES):
        halo = (
            np.zeros(128, np.float32) if m == 0 else r[m * C - 128 : m * C]
        )
        rbuf = np.concatenate([halo, r[m * C : (m + 1) * C]])
        R = np.ascontiguousarray(rbuf.reshape(J + 1, 128).T)
        in_maps.append({"R": R, "W0": W0, "W1": W1})

    nc = _get_program(MM_DT)
    res = run_bass_kernel_spmd(
        nc, in_maps, core_ids=list(range(N_CORES)), trace=_trace, **_trace_kwargs
    )
    LAST_RESULTS = res

    y = np.concatenate(
        [
            np.ascontiguousarray(res.results[m]["Y"].T).reshape(-1)
            for m in range(N_CORES)
        ]
    )
    return y


# revision 11
# speedup vs baseline: 1.3098x; 1.1999x over previous
"""Discrete transfer function (2nd-order IIR, Butterworth lowpass) over T=2^23
samples, on 8 TRN2 NeuronCores.

Approach: the filter is stable (poles at radius ~0.577), so the IIR's impulse
response decays below float32 precision within ~100 taps.  The whole recurrence
y = filter(b, a, r), shifted by one sample and scaled by dt, is therefore
exactly (to f32 precision) a 128-tap FIR convolution y[t] = sum_n g[n] r[t-n],
with g derived from (b, a) on the host in float64.

The convolution maps onto the TensorEngine as two banded-Toeplitz matmuls:
with R[k, j] = r[128 j + k] (a [128, J] reshape of the signal),
    Y[c, j] = sum_k W0[k, c] R[k, j] + sum_k W1[k, c] R[k, j-1]
where W0[k, c] = g[c - k] (lower-triangular band) and W1[k, c] = g[c - k + 128]
(strictly upper triangle).  Each core gets a contiguous 2^20-sample chunk plus
a 128-sample halo column from its left neighbor.  The host does the (free)
reshape/transpose so every DMA is 128 partitions x contiguous rows.

Raw bass (not Tile): the pipeline is static and simple — input DMAs on the SP
HWDGE ring, matmuls on PE, PSUM->SBUF copies on DVE, output DMAs on the ACT
HWDGE ring, double-buffered with explicit semaphores.  (Tile was tried first
but attaches >1 embedded sync-wait to Matmult instructions, which the walrus
codegen rejects; standalone WAIT instructions have no such limit.)
"""

import numpy as np

import concourse.bass as bass
import concourse.mybir as mybir
from concourse.bass_utils import run_bass_kernel_spmd

N_CORES = 8
T = 8388608                 # 2**23, matches the fixed problem size
C = T // N_CORES            # samples per core
J = C // 128                # columns per core in the [128, J] layout
BLK = 1024                  # columns per pipeline block (512 KiB per DMA)
NBLK = J // BLK
L = 128                     # FIR taps kept (g[n] ~ 0.577^n -> ~1e-30 at n=127)
DT = 1.0                    # module's dt (hardcoded in the source nn.Module)
CP_SPLIT = 512              # PSUM->SBUF drain split (bank-aligned): DVE [0,512), ACT [512,BLK)

# Matmul operand dtype: float32 is exact; float32r runs the PE 4x faster at
# ~TF32-ish precision.
MM_DT = mybir.dt.float32r

_PROGRAM_CACHE: dict = {}

# test.py reads this after calling kernel(..., _trace=True)
LAST_RESULTS = None


def _fir_coeffs(b: np.ndarray, a: np.ndarray) -> np.ndarray:
    """g[n] (length L, float64) with y_out[t] = sum_n g[n] r[t-n].

    g folds together: the IIR impulse response of b/a, the module's
    one-sample output delay, and the dt scale."""
    b = np.asarray(b, np.float64)
    a = np.asarray(a, np.float64)
    a = a / a[0]
    h = np.zeros(L, np.float64)
    for n in range(L):
        acc = b[n] if n < len(b) else 0.0
        for j in range(1, len(a)):
            if n - j >= 0:
                acc -= a[j] * h[n - j]
        h[n] = acc
    g = np.zeros(L, np.float64)
    g[1:] = DT * h[: L - 1]  # y_out[t] = dt * y_iir[t-1], y_out[0] = 0
    return g


def _weight_mats(g: np.ndarray) -> tuple[np.ndarray, np.ndarray]:
    g32 = g.astype(np.float32)
    idx_c = np.arange(128)[None, :]
    idx_k = np.arange(128)[:, None]
    d = idx_c - idx_k  # c - k
    W0 = np.where(d >= 0, g32[np.clip(d, 0, L - 1)], 0.0).astype(np.float32)
    W1 = np.where(d < 0, g32[np.clip(d + 128, 0, L - 1)], 0.0).astype(np.float32)
    return W0, W1


def _build_program(
    mm_dt,
    blk=None,
    nbuf=4,
    nblk=None,
    cp_split=None,
    pairwise=False,
    act_copy=True,
):
    """Static double(4x)-buffered pipeline.

    Per-slot semaphores: each rin/yout slot has its own DMA-completion
    semaphore, so a wait can never be satisfied by a mixture of partial
    completions from different transfers (the 16 SDMA engines post 16
    independent +1s per transfer, and per-engine FIFO only orders
    *per-engine* — a cumulative shared counter races)."""
    blk = BLK if blk is None else blk
    nblk = (J // blk) if nblk is None else nblk
    cp_split = CP_SPLIT if cp_split is None else cp_split
    if not act_copy:
        cp_split = blk
    f32 = mybir.dt.float32
    nc = bass.Bass()
    R = nc.dram_tensor("R", [128, J + 1], mm_dt, kind="ExternalInput")
    W0 = nc.dram_tensor("W0", [128, 128], mm_dt, kind="ExternalInput")
    W1 = nc.dram_tensor("W1", [128, 128], mm_dt, kind="ExternalInput")
    Y = nc.dram_tensor("Y", [128, J], f32, kind="ExternalOutput")

    import contextlib

    with contextlib.ExitStack() as ctx:
        w0t = ctx.enter_context(nc.sbuf_tensor([128, 128], mm_dt))
        w1t = ctx.enter_context(nc.sbuf_tensor([128, 128], mm_dt))
        rin_all = ctx.enter_context(
            nc.sbuf_tensor([128, nbuf * (blk + 1)], mm_dt)
        )
        yout_all = ctx.enter_context(nc.sbuf_tensor([128, nbuf * blk], f32))
        pts = [
            ctx.enter_context(nc.psum_tensor(f"pt{i}", [128, blk], f32))
            for i in range(nbuf)
        ]
        s_w = ctx.enter_context(nc.semaphore("s_w"))
        s_in = [
            ctx.enter_context(nc.semaphore(f"s_in{i}")) for i in range(nbuf)
        ]
        s_out = [
            ctx.enter_context(nc.semaphore(f"s_out{i}")) for i in range(nbuf)
        ]
        s_pe = ctx.enter_context(nc.semaphore("s_pe"))
        s_cpv = ctx.enter_context(nc.semaphore("s_cpv"))
        s_cpa = ctx.enter_context(nc.semaphore("s_cpa"))
        block = ctx.enter_context(nc.Block())

        rin = [rin_all[:, i * (blk + 1) : (i + 1) * (blk + 1)] for i in range(nbuf)]
        yout = [yout_all[:, i * blk : (i + 1) * blk] for i in range(nbuf)]

        @block.sync
        def _(sync):
            sync.dma_start(out=w0t[:, :], in_=W0[:, :]).then_inc(s_w, 16)
            sync.dma_start(out=w1t[:, :], in_=W1[:, :]).then_inc(s_w, 16)
            for b_ in range(nblk):
                i = b_ % nbuf
                if b_ >= nbuf:
                    # rin slot reuse: matmuls of block b_-nbuf must be done
                    sync.wait_ge(s_pe, b_ - nbuf + 1)
                sync.dma_start(
                    out=rin[i],
                    in_=R[:, b_ * blk : b_ * blk + blk + 1],
                ).then_inc(s_in[i], 16)

        @block.tensor
        def _(tensor):
            tensor.wait_ge(s_w, 32)
            for b_ in range(nblk):
                i = b_ % nbuf
                tensor.wait_ge(s_in[i], 16 * (b_ // nbuf + 1))
                if b_ >= nbuf:
                    # psum slot drained by both copy engines
                    tensor.wait_ge(s_cpv, b_ - nbuf + 1)
                    if act_copy:
                        tensor.wait_ge(s_cpa, b_ - nbuf + 1)
                pt = pts[i]
                ri = rin[i]
                nsub = blk // 512
                if pairwise:
                    for s in range(nsub):
                        nc.tensor.matmul(
                            pt[:, s * 512 : (s + 1) * 512],
                            w0t[:, :],
                            ri[:, s * 512 + 1 : s * 512 + 513],
                            start=True,
                            stop=False,
                        )
                        mm = nc.tensor.matmul(
                            pt[:, s * 512 : (s + 1) * 512],
                            w1t[:, :],
                            ri[:, s * 512 : s * 512 + 512],
                            start=False,
                            stop=True,
                        )
                else:
                    for s in range(nsub):
                        nc.tensor.matmul(
                            pt[:, s * 512 : (s + 1) * 512],
                            w0t[:, :],
                            ri[:, s * 512 + 1 : s * 512 + 513],
                            start=True,
                            stop=False,
                        )
                    for s in range(nsub):
                        mm = nc.tensor.matmul(
                            pt[:, s * 512 : (s + 1) * 512],
                            w1t[:, :],
                            ri[:, s * 512 : s * 512 + 512],
                            start=False,
                            stop=True,
                        )
                mm.then_inc(s_pe, 1)

        @block.vector
        def _(vector):
            for b_ in range(nblk):
                i = b_ % nbuf
                vector.wait_ge(s_pe, b_ + 1)
                if b_ >= nbuf:
                    vector.wait_ge(s_out[i], 16 * (b_ // nbuf))  # yout flushed
                nc.vector.tensor_copy(
                    out=yout[i][:, :cp_split],
                    in_=pts[i][:, :cp_split],
                ).then_inc(s_cpv, 1)

        @block.scalar
        def _(scalar):
            for b_ in range(nblk):
                i = b_ % nbuf
                if act_copy:
                    scalar.wait_ge(s_pe, b_ + 1)
                    if b_ >= nbuf:
                        scalar.wait_ge(s_out[i], 16 * (b_ // nbuf))
                    nc.scalar.copy(
                        out=yout[i][:, cp_split:],
                        in_=pts[i][:, cp_split:],
                    ).then_inc(s_cpa, 1)
                scalar.wait_ge(s_cpv, b_ + 1)
                scalar.dma_start(
                    out=Y[:, b_ * blk : (b_ + 1) * blk], in_=yout[i]
                ).then_inc(s_out[i], 16)
            for i in range(nbuf):
                last_b = max(b_ for b_ in range(nblk) if b_ % nbuf == i)
                scalar.wait_ge(s_out[i], 16 * (last_b // nbuf + 1))

    return nc


def _get_program(mm_dt):
    key = str(mm_dt)
    if key not in _PROGRAM_CACHE:
        _PROGRAM_CACHE[key] = _build_program(mm_dt)
    return _PROGRAM_CACHE[key]


def kernel(r, b, a, _trace: bool = False, **_trace_kwargs):
    global LAST_RESULTS
    r = np.ascontiguousarray(np.asarray(r, dtype=np.float32))
    assert r.shape == (T,), r.shape

    g = _fir_coeffs(b, a)
    W0, W1 = _weight_mats(g)

    in_maps = []
    for m in range(N_CORES):
        halo = (
            np.zeros(128, np.float32) if m == 0 else r[m * C - 128 : m * C]
        )
        rbuf = np.concatenate([halo, r[m * C : (m + 1) * C]])
        R = np.ascontiguousarray(rbuf.reshape(J + 1, 128).T)
        in_maps.append({"R": R, "W0": W0, "W1": W1})

    nc = _get_program(MM_DT)
    res = run_bass_kernel_spmd(
        nc, in_maps, core_ids=list(range(N_CORES)), trace=_trace, **_trace_kwargs
    )
    LAST_RESULTS = res

    y = np.concatenate(
        [
            np.ascontiguousarray(res.results[m]["Y"].T).reshape(-1)
            for m in range(N_CORES)
        ]
    )
    return y


# revision 15
# speedup vs baseline: 1.3747x; 1.0496x over previous
"""Discrete transfer function (2nd-order IIR, Butterworth lowpass) over T=2^23
samples, on 8 TRN2 NeuronCores.

Approach: the filter is stable (poles at radius ~0.577), so the IIR's impulse
response decays below float32 precision within ~100 taps.  The whole recurrence
y = filter(b, a, r), shifted by one sample and scaled by dt, is therefore
exactly (to f32 precision) a 128-tap FIR convolution y[t] = sum_n g[n] r[t-n],
with g derived from (b, a) on the host in float64.

The convolution maps onto the TensorEngine as two banded-Toeplitz matmuls:
with R[k, j] = r[128 j + k] (a [128, J] reshape of the signal),
    Y[c, j] = sum_k W0[k, c] R[k, j] + sum_k W1[k, c] R[k, j-1]
where W0[k, c] = g[c - k] (lower-triangular band) and W1[k, c] = g[c - k + 128]
(strictly upper triangle).  Each core gets a contiguous 2^20-sample chunk plus
a 128-sample halo column from its left neighbor.  The host does the (free)
reshape/transpose so every DMA is 128 partitions x contiguous rows.

Raw bass (not Tile): the pipeline is static and simple — input DMAs on the SP
HWDGE ring, matmuls on PE, PSUM->SBUF copies on DVE, output DMAs on the ACT
HWDGE ring, double-buffered with explicit semaphores.  (Tile was tried first
but attaches >1 embedded sync-wait to Matmult instructions, which the walrus
codegen rejects; standalone WAIT instructions have no such limit.)
"""

import numpy as np

import concourse.bass as bass
import concourse.mybir as mybir
from concourse.bass_utils import run_bass_kernel_spmd

N_CORES = 8
T = 8388608                 # 2**23, matches the fixed problem size
C = T // N_CORES            # samples per core
J = C // 128                # columns per core in the [128, J] layout
BLK = 1024                  # columns per pipeline block (512 KiB per DMA)
NBLK = J // BLK
L = 128                     # FIR taps kept (g[n] ~ 0.577^n -> ~1e-30 at n=127)
DT = 1.0                    # module's dt (hardcoded in the source nn.Module)
CP_SPLIT = 512              # PSUM->SBUF drain split (bank-aligned): DVE [0,512), ACT [512,BLK)

# Matmul operand dtype.  float32 is exact but runs the PE at 4 cyc/row with a
# serialized per-matmul weight reload; float32r is ~TF32 precision (measured
# 1.6e-4 rel); float16 matches that precision (measured 2.5e-4 rel), halves
# input DMA traffic, and streams at 1 cyc/row with fast weight load.
MM_DT = mybir.dt.float16

_PROGRAM_CACHE: dict = {}

# test.py reads this after calling kernel(..., _trace=True)
LAST_RESULTS = None


def _fir_coeffs(b: np.ndarray, a: np.ndarray) -> np.ndarray:
    """g[n] (length L, float64) with y_out[t] = sum_n g[n] r[t-n].

    g folds together: the IIR impulse response of b/a, the module's
    one-sample output delay, and the dt scale."""
    b = np.asarray(b, np.float64)
    a = np.asarray(a, np.float64)
    a = a / a[0]
    h = np.zeros(L, np.float64)
    for n in range(L):
        acc = b[n] if n < len(b) else 0.0
        for j in range(1, len(a)):
            if n - j >= 0:
                acc -= a[j] * h[n - j]
        h[n] = acc
    g = np.zeros(L, np.float64)
    g[1:] = DT * h[: L - 1]  # y_out[t] = dt * y_iir[t-1], y_out[0] = 0
    return g


def _weight_mats(g: np.ndarray) -> tuple[np.ndarray, np.ndarray]:
    g32 = g.astype(np.float32)
    idx_c = np.arange(128)[None, :]
    idx_k = np.arange(128)[:, None]
    d = idx_c - idx_k  # c - k
    W0 = np.where(d >= 0, g32[np.clip(d, 0, L - 1)], 0.0).astype(np.float32)
    W1 = np.where(d < 0, g32[np.clip(d + 128, 0, L - 1)], 0.0).astype(np.float32)
    return W0, W1


def _build_program(
    mm_dt,
    blk=None,
    nbuf=4,
    nblk=None,
    cp_split=None,
    pairwise=False,
    act_copy=True,
):
    """Static double(4x)-buffered pipeline.

    Per-slot semaphores: each rin/yout slot has its own DMA-completion
    semaphore, so a wait can never be satisfied by a mixture of partial
    completions from different transfers (the 16 SDMA engines post 16
    independent +1s per transfer, and per-engine FIFO only orders
    *per-engine* — a cumulative shared counter races)."""
    blk = BLK if blk is None else blk
    nblk = (J // blk) if nblk is None else nblk
    cp_split = CP_SPLIT if cp_split is None else cp_split
    if not act_copy:
        cp_split = blk
    f32 = mybir.dt.float32
    nc = bass.Bass()
    R = nc.dram_tensor("R", [128, J + 1], mm_dt, kind="ExternalInput")
    W0 = nc.dram_tensor("W0", [128, 128], mm_dt, kind="ExternalInput")
    W1 = nc.dram_tensor("W1", [128, 128], mm_dt, kind="ExternalInput")
    Y = nc.dram_tensor("Y", [128, J], f32, kind="ExternalOutput")

    import contextlib

    with contextlib.ExitStack() as ctx:
        w0t = ctx.enter_context(nc.sbuf_tensor([128, 128], mm_dt))
        w1t = ctx.enter_context(nc.sbuf_tensor([128, 128], mm_dt))
        rin_all = ctx.enter_context(
            nc.sbuf_tensor([128, nbuf * (blk + 1)], mm_dt)
        )
        yout_all = ctx.enter_context(nc.sbuf_tensor([128, nbuf * blk], f32))
        pts = [
            ctx.enter_context(nc.psum_tensor(f"pt{i}", [128, blk], f32))
            for i in range(nbuf)
        ]
        s_w = ctx.enter_context(nc.semaphore("s_w"))
        s_in = [
            ctx.enter_context(nc.semaphore(f"s_in{i}")) for i in range(nbuf)
        ]
        s_out = [
            ctx.enter_context(nc.semaphore(f"s_out{i}")) for i in range(nbuf)
        ]
        s_pe = ctx.enter_context(nc.semaphore("s_pe"))
        s_cpv = ctx.enter_context(nc.semaphore("s_cpv"))
        s_cpa = ctx.enter_context(nc.semaphore("s_cpa"))
        block = ctx.enter_context(nc.Block())

        rin = [rin_all[:, i * (blk + 1) : (i + 1) * (blk + 1)] for i in range(nbuf)]
        yout = [yout_all[:, i * blk : (i + 1) * blk] for i in range(nbuf)]

        @block.sync
        def _(sync):
            for b_ in range(nblk):
                i = b_ % nbuf
                if b_ >= nbuf:
                    # rin slot reuse: matmuls of block b_-nbuf must be done
                    sync.wait_ge(s_pe, b_ - nbuf + 1)
                sync.dma_start(
                    out=rin[i],
                    in_=R[:, b_ * blk : b_ * blk + blk + 1],
                ).then_inc(s_in[i], 16)
                if b_ == 0:
                    # weights after the first (critical-path) block load
                    sync.dma_start(out=w0t[:, :], in_=W0[:, :]).then_inc(s_w, 16)
                    sync.dma_start(out=w1t[:, :], in_=W1[:, :]).then_inc(s_w, 16)

        @block.tensor
        def _(tensor):
            tensor.wait_ge(s_w, 32)
            for b_ in range(nblk):
                i = b_ % nbuf
                tensor.wait_ge(s_in[i], 16 * (b_ // nbuf + 1))
                if b_ >= nbuf:
                    # psum slot drained by both copy engines
                    tensor.wait_ge(s_cpv, b_ - nbuf + 1)
                    if act_copy:
                        tensor.wait_ge(s_cpa, b_ - nbuf + 1)
                pt = pts[i]
                ri = rin[i]
                nsub = blk // 512
                if pairwise:
                    for s in range(nsub):
                        nc.tensor.matmul(
                            pt[:, s * 512 : (s + 1) * 512],
                            w0t[:, :],
                            ri[:, s * 512 + 1 : s * 512 + 513],
                            start=True,
                            stop=False,
                        )
                        mm = nc.tensor.matmul(
                            pt[:, s * 512 : (s + 1) * 512],
                            w1t[:, :],
                            ri[:, s * 512 : s * 512 + 512],
                            start=False,
                            stop=True,
                        )
                else:
                    for s in range(nsub):
                        nc.tensor.matmul(
                            pt[:, s * 512 : (s + 1) * 512],
                            w0t[:, :],
                            ri[:, s * 512 + 1 : s * 512 + 513],
                            start=True,
                            stop=False,
                        )
                    for s in range(nsub):
                        mm = nc.tensor.matmul(
                            pt[:, s * 512 : (s + 1) * 512],
                            w1t[:, :],
                            ri[:, s * 512 : s * 512 + 512],
                            start=False,
                            stop=True,
                        )
                mm.then_inc(s_pe, 1)

        @block.vector
        def _(vector):
            for b_ in range(nblk):
                i = b_ % nbuf
                vector.wait_ge(s_pe, b_ + 1)
                if b_ >= nbuf:
                    vector.wait_ge(s_out[i], 16 * (b_ // nbuf))  # yout flushed
                nc.vector.tensor_copy(
                    out=yout[i][:, :cp_split],
                    in_=pts[i][:, :cp_split],
                ).then_inc(s_cpv, 1)

        # Output DMAs alternate between the ACT HWDGE ring (even blocks) and
        # the GPSIMD SWDGE path (odd blocks) so the output stream isn't
        # limited by a single queue's throughput.
        def _out_blocks(parity):
            return [b_ for b_ in range(nblk) if b_ % 2 == parity]

        def _final_waits(engine, blocks):
            per_slot = {}
            for b_ in blocks:
                per_slot[b_ % nbuf] = b_
            for i, last_b in per_slot.items():
                engine.wait_ge(s_out[i], 16 * (last_b // nbuf + 1))

        @block.scalar
        def _(scalar):
            for b_ in range(nblk):
                i = b_ % nbuf
                if act_copy:
                    scalar.wait_ge(s_pe, b_ + 1)
                    if b_ >= nbuf:
                        scalar.wait_ge(s_out[i], 16 * (b_ // nbuf))
                    nc.scalar.copy(
                        out=yout[i][:, cp_split:],
                        in_=pts[i][:, cp_split:],
                    ).then_inc(s_cpa, 1)
                if b_ % 2 == 0:
                    scalar.wait_ge(s_cpv, b_ + 1)
                    scalar.dma_start(
                        out=Y[:, b_ * blk : (b_ + 1) * blk], in_=yout[i]
                    ).then_inc(s_out[i], 16)
            _final_waits(scalar, _out_blocks(0))

        @block.gpsimd
        def _(gpsimd):
            for b_ in _out_blocks(1):
                i = b_ % nbuf
                gpsimd.wait_ge(s_cpv, b_ + 1)
                if act_copy:
                    gpsimd.wait_ge(s_cpa, b_ + 1)
                gpsimd.dma_start(
                    out=Y[:, b_ * blk : (b_ + 1) * blk], in_=yout[i]
                ).then_inc(s_out[i], 16)
            _final_waits(gpsimd, _out_blocks(1))

    return nc


def _get_program(mm_dt):
    key = str(mm_dt)
    if key not in _PROGRAM_CACHE:
        _PROGRAM_CACHE[key] = _build_program(mm_dt)
    return _PROGRAM_CACHE[key]


def kernel(r, b, a, _trace: bool = False, **_trace_kwargs):
    global LAST_RESULTS
    r = np.ascontiguousarray(np.asarray(r, dtype=np.float32))
    assert r.shape == (T,), r.shape

    g = _fir_coeffs(b, a)
    W0, W1 = _weight_mats(g)

    np_dt = mybir.dt.np(MM_DT)  # float16 for 2-byte operands, else float32
    W0 = W0.astype(np_dt)
    W1 = W1.astype(np_dt)

    in_maps = []
    for m in range(N_CORES):
        halo = (
            np.zeros(128, np.float32) if m == 0 else r[m * C - 128 : m * C]
        )
        rbuf = np.concatenate([halo, r[m * C : (m + 1) * C]]).astype(np_dt)
        R = np.ascontiguousarray(rbuf.reshape(J + 1, 128).T)
        in_maps.append({"R": R, "W0": W0, "W1": W1})

    nc = _get_program(MM_DT)
    res = run_bass_kernel_spmd(
        nc, in_maps, core_ids=list(range(N_CORES)), trace=_trace, **_trace_kwargs
    )
    LAST_RESULTS = res

    y = np.concatenate(
        [
            np.ascontiguousarray(res.results[m]["Y"].T).reshape(-1)
            for m in range(N_CORES)
        ]
    )
    return y
